# revision 35
# baseline (speedup 1.0000x reference)
"""Trainium2 Bass kernel for nn_BinnedLoss (tent-weighted 128-bin chi2 loss).

Self-contained 8-core SPMD program, data-parallel over the N=16.7M sample
axis. Per core each array is [128, 16384] f32. The tent-weighted histogram
is fully determined by per-interval moments M0[k]=sum(w), M1[k]=sum(w*frac)
(k = clipped floor((x-mn)/step), frac = u-k), so instead of a per-bin
mask-reduce loop the kernel:
  * computes u on the Scalar engine, kc/kh/kl/frac via fused 2x/4x-mode
    tensor_scalar ops on DVE,
  * builds planar one-hot factors (kc = 16*kh + kl) in bf16 with 4x-mode
    is_equal tensor_scalar ops,
  * accumulates per-cell moments with block-diagonal [128,128] bf16 matmuls
    on the Tensor engine into PSUM (8 sample-columns per matmul; both
    operands contiguous via a grouped-interleaved plane layout, since the
    stationary operand must be a 1-free-dim AP and a multi-dim moving AP
    streams ~6x slower),
  * offloads part of the one-hot builds to the Scalar engine as
    Relu(1 - |k - c|) activation pairs,
  * extracts the 8 diagonal [16,16] blocks, allreduces one [1,256] vector
    per array (the sim one hides under exp compute), and assembles
    raw[b] = M1[b-1] + M0[b] - M1[b] (b in 1..126) before the
    normalization and chi2 reduction.

kernel(**inputs) -> np.float32 scalar (shape ()).
"""
import os
import sys

sys.path.insert(0, "/opt/trn_rl_repo")
import numpy as np

N = 16777216
NCORES = 8
BINS = 128
P = 128
NSH = N // NCORES            # samples per core
FTOT = NSH // P              # 16384 free columns per core per array
FC = 1024                    # columns per phase-B chunk
NCH = FTOT // FC
MAGIC = 8388608.0            # 2^23
MHALF = 8388607.5            # 2^23 - 0.5 (round(u-0.5) == floor for our grid)


def _patches(mybir, tile):
    from concourse.vector_clock import ScopedClock

    def _patched(self, tick_clock, wait_clock):
        drain_inst = self.nc.sync.drain()
        wait_clock.add_sem_waits(
            drain_inst.ins, ScopedClock({None: tick_clock.global_clock})
        )
        si = drain_inst.ins.sync_info
        if si is not None and si.on_wait and len(si.on_wait) > 1:
            waits = list(si.on_wait)
            drain_inst.ins.sync_info = mybir.SyncInfo(
                on_wait=[waits[0]], on_update=list(si.on_update)
            )
            for w in waits[1:]:
                nop = self.nc.sync.nop()
                nop.ins.sync_info = mybir.SyncInfo(on_wait=[w], on_update=[])
        self.nc.all_engine_barrier()
        assert self.sems is not None
        popped = self.nc._tile_sem_poison_stack.pop()
        assert popped is self._sem_poison
        self.nc.clear_and_free_semaphores(list(self.sems.allocated().values()))
        self.nc.all_engine_barrier()

    tile.TileContext._drain_and_barrier = _patched


def _split_sync_waits(nc, mybir, strip_same_engine=True):
    """Two fixups for this walrus/runtime:
    1. Drop same-engine waits (engines are in-order; engine-vs-own-sem waits
       are redundant) -- wait-carrying instructions are ~10x slower here.
    2. The walrus build allows <=1 sem-wait per instruction; hoist extras
       onto same-engine NOPs inserted just before the instruction."""
    eng_sem = {}
    counter = [0]
    for f in nc.m.functions:
        for bb in f.blocks:
            out = []
            dirty = False
            for inst in bb.instructions:
                si = inst.sync_info
                pref = eng_sem.get(inst.engine) if strip_same_engine else None
                if si is not None and si.on_wait and pref is not None:
                    kept = [
                        w for w in si.on_wait
                        if not (w.ant_name or "").startswith(pref + "_")
                    ]
                    if len(kept) != len(si.on_wait):
                        inst.sync_info = mybir.SyncInfo(
                            on_wait=kept, on_update=list(si.on_update))
                        si = inst.sync_info
                        dirty = True
                if si is not None and si.on_wait and len(si.on_wait) > 1:
                    waits = list(si.on_wait)
                    for w in waits[:-1]:
                        counter[0] += 1
                        nop = mybir.InstNoOp(
                            name=f"WSPLIT-{counter[0]}", ins=[], outs=[]
                        )
                        nop.engine = inst.engine
                        nop.sync_info = mybir.SyncInfo(on_wait=[w], on_update=[])
                        nc.register_instruction(nop, overwrite=True)
                        out.append(nop)
                    inst.sync_info = mybir.SyncInfo(
                        on_wait=[waits[-1]], on_update=list(si.on_update)
                    )
                    dirty = True
                out.append(inst)
            if dirty:
                bb.instructions = out


def build(ncores=NCORES):
    import concourse.bass as bass
    import concourse.mybir as mybir
    from concourse import tile

    _patches(mybir, tile)
    DT = mybir.dt
    AL = mybir.AluOpType
    ACT = mybir.ActivationFunctionType
    F32 = DT.float32
    BF16 = DT.bfloat16
    core_ids = list(range(ncores))

    nc = bass.Bass()
    sim_ext = nc.declare_dram_parameter("sim", [P, FTOT], F32, isOutput=False)
    exp_ext = nc.declare_dram_parameter("exp", [P, FTOT], F32, isOutput=False)
    w_ext = nc.declare_dram_parameter("w", [P, FTOT], F32, isOutput=False)
    out_ext = nc.declare_dram_parameter("out", [1, 1], F32, isOutput=True)

    with tile.TileContext(nc) as tc:
        with (
            tc.tile_pool(name="const", bufs=1) as cpool,
            tc.tile_pool(name="dram", bufs=1, space="DRAM") as dram,
            tc.tile_pool(name="psum", bufs=1, space="PSUM") as psum,
        ):
            cc_a_in = dram.tile([1, 2], F32, name="cc_a_in")
            cc_a_out = dram.tile([1, 2], F32, name="cc_a_out")
            cc_h_in = [dram.tile([1, 256], F32, name=f"cc_h_in{a}")
                       for a in range(2)]
            cc_h_out = [dram.tile([1, 256], F32, name=f"cc_h_out{a}")
                        for a in range(2)]

            ones1 = cpool.tile([1, P], F32, name="ones1")
            nc.vector.memset(ones1[:], 1.0)

            # scalars: sc = [mn, inv, b_u, idel2] (+ scratch)
            sc = cpool.tile([1, 8], F32, name="sc")
            bc = cpool.tile([P, 2], F32, name="bc")
            bcps = psum.tile([P, 2], F32, name="bcps", tag="bcps")

            # per-array moment rows [M0 | M1] c-ordered, both arrays
            G = cpool.tile([1, 512], F32, name="G")
            GH = cpool.tile([1, 512], F32, name="GH")

            # ---------------- Phase A: global min/max ----------------
            with tc.tile_pool(name="pa", bufs=1) as pa:
                CW = 8192
                rmin = pa.tile([P, 1], F32, name="rmin")
                rmax = pa.tile([P, 1], F32, name="rmax")
                nc.vector.memset(rmin[:], 1.0e30)
                nc.vector.memset(rmax[:], -1.0e30)
                ch = pa.tile([P, CW], F32, name="ch", bufs=4)
                tmin = pa.tile([P, 1], F32, name="tmin")
                tmax = pa.tile([P, 1], F32, name="tmax")
                H = CW // 2
                for arr in (sim_ext, exp_ext):
                    for cv in range(0, FTOT, CW):
                        nc.sync.dma_start(ch[:, 0:H], arr[:, cv:cv + H])
                        nc.scalar.dma_start(
                            ch[:, H:CW], arr[:, cv + H:cv + CW])
                        nc.vector.tensor_reduce(
                            tmin[:], ch[:], mybir.AxisListType.X, AL.min)
                        nc.vector.tensor_reduce(
                            tmax[:], ch[:], mybir.AxisListType.X, AL.max)
                        nc.vector.tensor_tensor(
                            rmin[:], rmin[:], tmin[:], AL.min)
                        nc.vector.tensor_tensor(
                            rmax[:], rmax[:], tmax[:], AL.max)
                pm = pa.tile([1, 2 * P], F32, name="pm")
                nc.gpsimd.dma_start(pm[0:1, 0:P], rmax[:, 0:1])
                nc.gpsimd.dma_start(pm[0:1, P:2 * P], rmin[:, 0:1])
                pk = pa.tile([1, 2], F32, name="pk")
                nc.vector.tensor_reduce(
                    pk[0:1, 0:1], pm[0:1, 0:P], mybir.AxisListType.X, AL.max)
                nc.vector.tensor_reduce(
                    pk[0:1, 1:2], pm[0:1, P:2 * P], mybir.AxisListType.X,
                    AL.min)
                nc.vector.tensor_scalar_mul(pk[0:1, 1:2], pk[0:1, 1:2], -1.0)
                nc.gpsimd.dma_start(cc_a_in[:], pk[:])
                nc.gpsimd.collective_compute(
                    "AllReduce", AL.max, replica_groups=[core_ids],
                    ins=[cc_a_in.opt()], outs=[cc_a_out.opt()],
                )
                ga = pa.tile([1, 2], F32, name="ga")
                nc.gpsimd.dma_start(ga[:], cc_a_out[:])
                # ga = [mx, -mn]
                nc.vector.tensor_scalar_mul(sc[0:1, 0:1], ga[0:1, 1:2], -1.0)
                d_t = pa.tile([1, 1], F32, name="d_t")
                nc.vector.tensor_tensor(
                    d_t[:], ga[0:1, 0:1], sc[0:1, 0:1], AL.subtract)
                # step = d/127 ; inv = 1/step ; b_u = -mn*inv
                stp = pa.tile([1, 1], F32, name="stp")
                nc.vector.tensor_scalar_mul(
                    stp[:], d_t[:], float(np.float32(1.0) / np.float32(127.0)))
                nc.vector.reciprocal(sc[0:1, 1:2], stp[:])
                nc.vector.scalar_tensor_tensor(
                    sc[0:1, 2:3], sc[0:1, 0:1], -1.0, sc[0:1, 1:2],
                    AL.mult, AL.mult)
                # idel2 = (1/delta)^2, delta = d/128
                dl = pa.tile([1, 1], F32, name="dl")
                nc.vector.tensor_scalar_mul(dl[:], d_t[:], 0.0078125)
                nc.vector.reciprocal(dl[:], dl[:])
                nc.vector.tensor_tensor(sc[0:1, 3:4], dl[:], dl[:], AL.mult)
                # broadcast [inv, b_u] to all partitions
                nc.tensor.matmul(bcps[:], ones1[:], sc[0:1, 1:3],
                                 start=True, stop=True)
                nc.vector.tensor_copy(bc[:], bcps[:])

            # ---------------- Phase B: moment accumulation ----------------
            with tc.tile_pool(name="pb", bufs=1) as pb:
                x = pb.tile([P, FC], F32, name="x", bufs=3)
                u = pb.tile([P, FC], F32, name="u", bufs=2)
                kcm = pb.tile([P, FC], F32, name="kcm")
                kcc = pb.tile([P, FC], F32, name="kcc")
                frac = pb.tile([P, FC], BF16, name="frac", bufs=2)
                khb = pb.tile([P, FC], BF16, name="khb", bufs=2)
                klb = pb.tile([P, FC], BF16, name="klb", bufs=2)
                AL16 = pb.tile([P, 16 * FC], BF16, name="AL16", bufs=2)
                RH = pb.tile([P, 16 * FC], BF16, name="RH", bufs=2)
                wt = pb.tile([P, FC], F32, name="wt", bufs=3)
                wb = pb.tile([P, FC], BF16, name="wb", bufs=2)
                ab8 = pb.tile([P, 8 * FC], BF16, name="ab8")
                hbias = pb.tile([P, 9], F32, name="hbias")
                lbias = pb.tile([P, 5], F32, name="lbias")
                for l in range(5):
                    nc.vector.memset(lbias[:, l:l + 1], -float(l))
                for h in range(8):
                    nc.vector.memset(hbias[:, h:h + 1], -float(h))
                nc.vector.memset(hbias[:, 8:9], 1.0)
                CC = pb.tile([P, 128], F32, name="CC", bufs=2)
                T = pb.tile([16, 128], F32, name="T", bufs=2)
                M = pb.tile([16, 16], F32, name="M", bufs=2)
                # grouped-interleave: AL16 mem idx = g*128 + l*8 + t
                ALg = AL16[:].rearrange("p (g l t) -> p l g t", l=16, t=8)
                # RH mem idx = g*128 + (m*8+h)*8 + t
                RHg = RH[:].rearrange("p (g n t) -> p n g t", n=16, t=8)
                ps_t = [psum.tile([P, 128], F32, name=f"ps{a}",
                                  tag=f"ps{a}") for a in range(2)]

                def extract(ai):
                    ps = ps_t[ai]
                    # extract: psum cell ps[8l+t, 8*(m*8+h)+t]
                    nc.vector.tensor_copy(CC[:], ps[:])
                    for t in range(8):
                        qe = nc.gpsimd if t % 2 == 0 else nc.sync
                        qe.dma_start(
                            T[0:16, 16 * t:16 * t + 16],
                            CC[t:128:8, t:128:8])
                    nc.vector.tensor_reduce(
                        M[:], T[0:16, :].rearrange("p (t n) -> p n t", t=8),
                        mybir.AxisListType.X, AL.add)
                    # G[0, 256*ai + 128*m + 16*h + l] = M[l, 8*m + h]
                    for m in range(2):
                        for h in range(8):
                            s = 256 * ai + 128 * m + 16 * h
                            qe = nc.gpsimd if h % 2 == 0 else nc.sync
                            qe.dma_start(
                                G[0:1, s:s + 16],
                                M[0:16, 8 * m + h:8 * m + h + 1])
                    nc.gpsimd.dma_start(
                        cc_h_in[ai][:], G[0:1, 256 * ai:256 * ai + 256])
                    nc.gpsimd.collective_compute(
                        "AllReduce", AL.add, replica_groups=[core_ids],
                        ins=[cc_h_in[ai].opt()], outs=[cc_h_out[ai].opt()],
                    )
                    nc.gpsimd.dma_start(
                        GH[0:1, 256 * ai:256 * ai + 256], cc_h_out[ai][:])

                # interleave sim/exp chunks; exp gets a 3-chunk head start
                # so its extraction + allreduce hide under sim's tail
                HS = 3
                seq = [(1, ci) for ci in range(HS)]
                si, ei = 0, HS
                while si < NCH or ei < NCH:
                    if si < NCH:
                        seq.append((0, si)); si += 1
                    if ei < NCH:
                        seq.append((1, ei)); ei += 1
                done = [0, 0]
                for ai, ci in seq:
                    arr = sim_ext if ai == 0 else exp_ext
                    weighted = ai == 0
                    ps = ps_t[ai]
                    if True:
                        c0 = ci * FC
                        nc.sync.dma_start(x[:], arr[:, c0:c0 + FC])
                        if weighted:
                            nc.sync.dma_start(wt[:], w_ext[:, c0:c0 + FC])
                            nc.scalar.activation(
                                wb[:], wt[:], ACT.Identity, bias=0.0,
                                scale=1.0)
                        # u = x*inv + b_u   (Scalar engine)
                        nc.scalar.activation(
                            u[:], x[:], ACT.Identity,
                            bias=bc[:, 1:2], scale=bc[:, 0:1])
                        # kc = clip(round(u-0.5), 0, 126)  [fused magic]
                        nc.vector.tensor_scalar(
                            kcm[:], u[:], MHALF, MAGIC + 126.0,
                            AL.add, AL.min)
                        nc.vector.tensor_scalar(
                            kcc[:], kcm[:], -MAGIC, 0.0, AL.add, AL.max)
                        # frac = u - kc  (bf16)
                        nc.vector.tensor_tensor(
                            frac[:], u[:], kcc[:], AL.subtract)
                        # kh = round(kc/16 - 0.499) ; kl = kc - 16*kh
                        nc.vector.tensor_scalar(
                            kcm[:], kcc[:], 0.0625, -0.499, AL.mult, AL.add)
                        nc.vector.tensor_scalar(
                            khb[:], kcm[:], 12582912.0, -12582912.0,
                            AL.add, AL.add)
                        nc.vector.scalar_tensor_tensor(
                            klb[:], khb[:], -16.0, kcc[:], AL.mult, AL.add)
                        # one-hot planes (grouped layout): AL16[.,g,l,t]
                        nal = 3 if weighted else 0
                        for l in range(nal):
                            nc.scalar.activation(
                                ab8[:, l * FC:(l + 1) * FC], klb[:],
                                ACT.Abs, bias=lbias[:, l:l + 1], scale=1.0)
                        for l in range(nal):
                            nc.scalar.activation(
                                ALg[:, l], ab8[:, l * FC:(l + 1) * FC],
                                ACT.Relu, bias=hbias[:, 8:9], scale=-1.0)
                        for l in range(nal, 16):
                            nc.vector.tensor_scalar(
                                ALg[:, l], klb[:],
                                float(l), 1.0, AL.is_equal, AL.mult)
                        # RH slot h = (kh == h): some planes on Scalar via
                        # Relu(1 - |kh - h|)  (all Abs, then all Relu)
                        nsc = 4 if weighted else 6
                        for h in range(nsc):
                            nc.scalar.activation(
                                ab8[:, h * FC:(h + 1) * FC], khb[:],
                                ACT.Abs, bias=hbias[:, h:h + 1], scale=1.0)
                        for h in range(nsc):
                            nc.scalar.activation(
                                RHg[:, h], ab8[:, h * FC:(h + 1) * FC],
                                ACT.Relu, bias=hbias[:, 8:9], scale=-1.0)
                        for h in range(nsc, 8):
                            nc.vector.tensor_scalar(
                                RHg[:, h], khb[:],
                                float(h), 1.0, AL.is_equal, AL.mult)
                        if weighted:
                            for h in range(8):
                                nc.vector.tensor_tensor(
                                    RHg[:, h], RHg[:, h], wb[:], AL.mult)
                        for h in range(8):
                            nc.vector.tensor_tensor(
                                RHg[:, 8 + h], RHg[:, h], frac[:], AL.mult)
                        # block-diagonal matmuls: 8 sample-cols per instr
                        for g in range(0, FC, 8):
                            first = ci == 0 and g == 0
                            last = ci == NCH - 1 and g == FC - 8
                            nc.tensor.matmul(
                                ps[:], AL16[:, g * 16:g * 16 + 128],
                                RH[:, g * 16:g * 16 + 128],
                                start=first, stop=last,
                                skip_group_check=True)
                    done[ai] += 1
                    if done[ai] == NCH:
                        extract(ai)

            # ---------------- Phase C: all-reduce + chi2 ----------------
            with tc.tile_pool(name="pc", bufs=1) as pc:
                R = pc.tile([1, 256], F32, name="R")
                nc.vector.memset(R[:], 0.0)
                for ai in range(2):
                    M0 = GH[0:1, 256 * ai:256 * ai + 128]
                    M1 = GH[0:1, 256 * ai + 128:256 * ai + 256]
                    Ra = R[0:1, 128 * ai:128 * ai + 128]
                    # raw[b] = M1[b-1] + M0[b] - M1[b],  b in 1..126
                    nc.vector.tensor_tensor(
                        Ra[0:1, 1:127], M0[0:1, 1:127], M1[0:1, 1:127],
                        AL.subtract)
                    nc.vector.tensor_tensor(
                        Ra[0:1, 1:127], Ra[0:1, 1:127], M1[0:1, 0:126],
                        AL.add)
                    ssum = pc.tile([1, 1], F32, name=f"ssum{ai}")
                    nc.vector.tensor_reduce(
                        ssum[:], Ra, mybir.AxisListType.X, AL.add)
                    nc.vector.reciprocal(ssum[:], ssum[:])
                    nc.vector.tensor_scalar(
                        Ra, Ra, ssum[0:1, 0:1], None, AL.mult)
                dif = pc.tile([1, BINS], F32, name="dif")
                nc.vector.tensor_tensor(
                    dif[:], R[0:1, 0:128], R[0:1, 128:256], AL.subtract)
                nc.vector.tensor_tensor(dif[:], dif[:], dif[:], AL.mult)
                chi = pc.tile([1, 1], F32, name="chi")
                nc.vector.tensor_reduce(
                    chi[:], dif[:], mybir.AxisListType.X, AL.add)
                # * (1/delta)^2
                nc.vector.tensor_scalar(
                    chi[:], chi[:], sc[0:1, 3:4], None, AL.mult)
                nc.gpsimd.dma_start(out_ext[:], chi[:])

    _split_sync_waits(nc, __import__("concourse.mybir", fromlist=["x"]),
                      strip_same_engine=True)
    return nc


_CACHE = {}


def _get_nc(repeat=1):
    key = (repeat,)
    if key not in _CACHE:
        _CACHE[key] = build()
    return _CACHE[key]


def make_in_maps(sim, exp, w):
    sim_s = sim.reshape(NCORES, P, FTOT)
    exp_s = exp.reshape(NCORES, P, FTOT)
    w_s = w.reshape(NCORES, P, FTOT)
    return [
        {"sim": sim_s[c], "exp": exp_s[c], "w": w_s[c]} for c in range(NCORES)
    ]


def finish(res):
    val = res.results[0]["out"][0, 0]
    return np.asarray(val, dtype=np.float32).reshape(())


def kernel(**inputs):
    sim = np.ascontiguousarray(inputs["sim_observable"], dtype=np.float32)
    exp = np.ascontiguousarray(inputs["exp_observable"], dtype=np.float32)
    w = np.ascontiguousarray(inputs["weights"], dtype=np.float32)
    assert sim.shape == (N,) and exp.shape == (N,) and w.shape == (N,)

    from concourse.bass_utils import run_bass_kernel_spmd

    repeat = int(os.environ.get("BASS_HIST_REPEAT", "1"))
    nc = _get_nc(repeat)
    res = run_bass_kernel_spmd(nc, make_in_maps(sim, exp, w),
                               list(range(NCORES)))
    return finish(res)


# revision 37
# speedup vs baseline: 1.2343x; 1.2343x over previous
"""Trainium2 Bass kernel for nn_BinnedLoss (tent-weighted 128-bin chi2 loss).

Self-contained 8-core SPMD program, data-parallel over the N=16.7M sample
axis. Per core each array is [128, 16384] f32. The tent-weighted histogram
is fully determined by per-interval moments M0[k]=sum(w), M1[k]=sum(w*frac)
(k = clipped floor((x-mn)/step), frac = u-k), so instead of a per-bin
mask-reduce loop the kernel:
  * computes u on the Scalar engine, kc/kh/kl/frac via fused 2x/4x-mode
    tensor_scalar ops on DVE,
  * builds planar one-hot factors (kc = 16*kh + kl) in bf16 with 4x-mode
    is_equal tensor_scalar ops,
  * accumulates per-cell moments with block-diagonal [128,128] bf16 matmuls
    on the Tensor engine into PSUM (8 sample-columns per matmul; both
    operands contiguous via a grouped-interleaved plane layout, since the
    stationary operand must be a 1-free-dim AP and a multi-dim moving AP
    streams ~6x slower),
  * offloads part of the one-hot builds to the Scalar engine as
    Relu(1 - |k - c|) activation pairs,
  * extracts the 8 diagonal [16,16] blocks, allreduces one [1,256] vector
    per array (the sim one hides under exp compute), and assembles
    raw[b] = M1[b-1] + M0[b] - M1[b] (b in 1..126) before the
    normalization and chi2 reduction.

kernel(**inputs) -> np.float32 scalar (shape ()).
"""
import os
import sys

sys.path.insert(0, "/opt/trn_rl_repo")
import numpy as np

N = 16777216
NCORES = 8
BINS = 128
P = 128
NSH = N // NCORES            # samples per core
FTOT = NSH // P              # 16384 free columns per core per array
FC = 1024                    # columns per phase-B chunk
NCH = FTOT // FC
MAGIC = 8388608.0            # 2^23
MHALF = 8388607.5            # 2^23 - 0.5 (round(u-0.5) == floor for our grid)


def _patches(mybir, tile):
    from concourse.vector_clock import ScopedClock

    def _patched(self, tick_clock, wait_clock):
        drain_inst = self.nc.sync.drain()
        wait_clock.add_sem_waits(
            drain_inst.ins, ScopedClock({None: tick_clock.global_clock})
        )
        si = drain_inst.ins.sync_info
        if si is not None and si.on_wait and len(si.on_wait) > 1:
            waits = list(si.on_wait)
            drain_inst.ins.sync_info = mybir.SyncInfo(
                on_wait=[waits[0]], on_update=list(si.on_update)
            )
            for w in waits[1:]:
                nop = self.nc.sync.nop()
                nop.ins.sync_info = mybir.SyncInfo(on_wait=[w], on_update=[])
        self.nc.all_engine_barrier()
        assert self.sems is not None
        popped = self.nc._tile_sem_poison_stack.pop()
        assert popped is self._sem_poison
        self.nc.clear_and_free_semaphores(list(self.sems.allocated().values()))
        self.nc.all_engine_barrier()

    tile.TileContext._drain_and_barrier = _patched


def _split_sync_waits(nc, mybir, strip_same_engine=True):
    """Two fixups for this walrus/runtime:
    1. Drop same-engine waits (engines are in-order; engine-vs-own-sem waits
       are redundant) -- wait-carrying instructions are ~10x slower here.
    2. The walrus build allows <=1 sem-wait per instruction; hoist extras
       onto same-engine NOPs inserted just before the instruction."""
    eng_sem = {}
    counter = [0]
    for f in nc.m.functions:
        for bb in f.blocks:
            out = []
            dirty = False
            for inst in bb.instructions:
                si = inst.sync_info
                pref = eng_sem.get(inst.engine) if strip_same_engine else None
                if si is not None and si.on_wait and pref is not None:
                    kept = [
                        w for w in si.on_wait
                        if not (w.ant_name or "").startswith(pref + "_")
                    ]
                    if len(kept) != len(si.on_wait):
                        inst.sync_info = mybir.SyncInfo(
                            on_wait=kept, on_update=list(si.on_update))
                        si = inst.sync_info
                        dirty = True
                if si is not None and si.on_wait and len(si.on_wait) > 1:
                    waits = list(si.on_wait)
                    for w in waits[:-1]:
                        counter[0] += 1
                        nop = mybir.InstNoOp(
                            name=f"WSPLIT-{counter[0]}", ins=[], outs=[]
                        )
                        nop.engine = inst.engine
                        nop.sync_info = mybir.SyncInfo(on_wait=[w], on_update=[])
                        nc.register_instruction(nop, overwrite=True)
                        out.append(nop)
                    inst.sync_info = mybir.SyncInfo(
                        on_wait=[waits[-1]], on_update=list(si.on_update)
                    )
                    dirty = True
                out.append(inst)
            if dirty:
                bb.instructions = out


def build(ncores=NCORES):
    import concourse.bass as bass
    import concourse.mybir as mybir
    from concourse import tile

    _patches(mybir, tile)
    DT = mybir.dt
    AL = mybir.AluOpType
    ACT = mybir.ActivationFunctionType
    F32 = DT.float32
    BF16 = DT.bfloat16
    core_ids = list(range(ncores))

    nc = bass.Bass()
    sim_ext = nc.declare_dram_parameter("sim", [P, FTOT], F32, isOutput=False)
    exp_ext = nc.declare_dram_parameter("exp", [P, FTOT], F32, isOutput=False)
    w_ext = nc.declare_dram_parameter("w", [P, FTOT], F32, isOutput=False)
    out_ext = nc.declare_dram_parameter("out", [1, 1], F32, isOutput=True)

    with tile.TileContext(nc) as tc:
        with (
            tc.tile_pool(name="const", bufs=1) as cpool,
            tc.tile_pool(name="dram", bufs=1, space="DRAM") as dram,
            tc.tile_pool(name="psum", bufs=1, space="PSUM") as psum,
        ):
            cc_a_in = dram.tile([1, 2], F32, name="cc_a_in")
            cc_a_out = dram.tile([1, 2], F32, name="cc_a_out")
            cc_h_in = [dram.tile([1, 256], F32, name=f"cc_h_in{a}")
                       for a in range(2)]
            cc_h_out = [dram.tile([1, 256], F32, name=f"cc_h_out{a}")
                        for a in range(2)]

            ones1 = cpool.tile([1, P], F32, name="ones1")
            nc.vector.memset(ones1[:], 1.0)

            # scalars: sc = [mn, inv, b_u, idel2] (+ scratch)
            sc = cpool.tile([1, 8], F32, name="sc")
            bc = cpool.tile([P, 2], F32, name="bc")
            bcps = psum.tile([P, 2], F32, name="bcps", tag="bcps")

            # per-array moment rows [M0 | M1] c-ordered, both arrays
            G = cpool.tile([1, 512], F32, name="G")
            GH = cpool.tile([1, 512], F32, name="GH")

            # ---------------- Phase A: global min/max ----------------
            with tc.tile_pool(name="pa", bufs=1) as pa:
                CW = 8192
                rmin = pa.tile([P, 1], F32, name="rmin")
                rmax = pa.tile([P, 1], F32, name="rmax")
                nc.vector.memset(rmin[:], 1.0e30)
                nc.vector.memset(rmax[:], -1.0e30)
                ch = pa.tile([P, CW], F32, name="ch", bufs=4)
                tmin = pa.tile([P, 1], F32, name="tmin")
                tmax = pa.tile([P, 1], F32, name="tmax")
                H = CW // 2
                for arr in (sim_ext, exp_ext):
                    for cv in range(0, FTOT, CW):
                        nc.sync.dma_start(ch[:, 0:H], arr[:, cv:cv + H])
                        nc.scalar.dma_start(
                            ch[:, H:CW], arr[:, cv + H:cv + CW])
                        nc.vector.tensor_reduce(
                            tmin[:], ch[:], mybir.AxisListType.X, AL.min)
                        nc.vector.tensor_reduce(
                            tmax[:], ch[:], mybir.AxisListType.X, AL.max)
                        nc.vector.tensor_tensor(
                            rmin[:], rmin[:], tmin[:], AL.min)
                        nc.vector.tensor_tensor(
                            rmax[:], rmax[:], tmax[:], AL.max)
                pm = pa.tile([1, 2 * P], F32, name="pm")
                nc.gpsimd.dma_start(pm[0:1, 0:P], rmax[:, 0:1])
                nc.gpsimd.dma_start(pm[0:1, P:2 * P], rmin[:, 0:1])
                pk = pa.tile([1, 2], F32, name="pk")
                nc.vector.tensor_reduce(
                    pk[0:1, 0:1], pm[0:1, 0:P], mybir.AxisListType.X, AL.max)
                nc.vector.tensor_reduce(
                    pk[0:1, 1:2], pm[0:1, P:2 * P], mybir.AxisListType.X,
                    AL.min)
                nc.vector.tensor_scalar_mul(pk[0:1, 1:2], pk[0:1, 1:2], -1.0)
                nc.gpsimd.dma_start(cc_a_in[:], pk[:])
                nc.gpsimd.collective_compute(
                    "AllReduce", AL.max, replica_groups=[core_ids],
                    ins=[cc_a_in.opt()], outs=[cc_a_out.opt()],
                )
                ga = pa.tile([1, 2], F32, name="ga")
                nc.gpsimd.dma_start(ga[:], cc_a_out[:])
                # ga = [mx, -mn]
                nc.vector.tensor_scalar_mul(sc[0:1, 0:1], ga[0:1, 1:2], -1.0)
                d_t = pa.tile([1, 1], F32, name="d_t")
                nc.vector.tensor_tensor(
                    d_t[:], ga[0:1, 0:1], sc[0:1, 0:1], AL.subtract)
                # step = d/127 ; inv = 1/step ; b_u = -mn*inv
                stp = pa.tile([1, 1], F32, name="stp")
                nc.vector.tensor_scalar_mul(
                    stp[:], d_t[:], float(np.float32(1.0) / np.float32(127.0)))
                nc.vector.reciprocal(sc[0:1, 1:2], stp[:])
                nc.vector.scalar_tensor_tensor(
                    sc[0:1, 2:3], sc[0:1, 0:1], -1.0, sc[0:1, 1:2],
                    AL.mult, AL.mult)
                # idel2 = (1/delta)^2, delta = d/128
                dl = pa.tile([1, 1], F32, name="dl")
                nc.vector.tensor_scalar_mul(dl[:], d_t[:], 0.0078125)
                nc.vector.reciprocal(dl[:], dl[:])
                nc.vector.tensor_tensor(sc[0:1, 3:4], dl[:], dl[:], AL.mult)
                # broadcast [inv, b_u] to all partitions
                nc.tensor.matmul(bcps[:], ones1[:], sc[0:1, 1:3],
                                 start=True, stop=True)
                nc.vector.tensor_copy(bc[:], bcps[:])

            # ---------------- Phase B: moment accumulation ----------------
            with tc.tile_pool(name="pb", bufs=1) as pb:
                x = pb.tile([P, FC], F32, name="x", bufs=3)
                u = pb.tile([P, FC], F32, name="u", bufs=2)
                kcm = pb.tile([P, FC], F32, name="kcm")
                kcc = pb.tile([P, FC], F32, name="kcc")
                frac = pb.tile([P, FC], BF16, name="frac", bufs=2)
                khb = pb.tile([P, FC], BF16, name="khb", bufs=2)
                klb = pb.tile([P, FC], BF16, name="klb", bufs=2)
                FH = FC // 2
                AL16h = [pb.tile([P, 16 * FH], BF16, name=f"AL16{i}",
                                 bufs=2) for i in range(2)]
                RHh = [pb.tile([P, 16 * FH], BF16, name=f"RH{i}", bufs=2)
                       for i in range(2)]
                wt = pb.tile([P, FC], F32, name="wt", bufs=3)
                wb = pb.tile([P, FC], BF16, name="wb", bufs=2)
                ab8 = pb.tile([P, 8 * FC], BF16, name="ab8")
                hbias = pb.tile([P, 9], F32, name="hbias")
                lbias = pb.tile([P, 7], F32, name="lbias")
                for l in range(7):
                    nc.vector.memset(lbias[:, l:l + 1], -float(l))
                for h in range(8):
                    nc.vector.memset(hbias[:, h:h + 1], -float(h))
                nc.vector.memset(hbias[:, 8:9], 1.0)
                CC = pb.tile([P, 128], F32, name="CC", bufs=2)
                T = pb.tile([16, 128], F32, name="T", bufs=2)
                M = pb.tile([16, 16], F32, name="M", bufs=2)
                # grouped-interleave: mem idx = g*128 + l*8 + t (per half)
                ALgh = [t_[:].rearrange("p (g l t) -> p l g t", l=16, t=8)
                        for t_ in AL16h]
                RHgh = [t_[:].rearrange("p (g n t) -> p n g t", n=16, t=8)
                        for t_ in RHh]
                ps_t = [psum.tile([P, 128], F32, name=f"ps{a}",
                                  tag=f"ps{a}") for a in range(2)]

                def extract(ai):
                    ps = ps_t[ai]
                    # extract: psum cell ps[8l+t, 8*(m*8+h)+t]
                    nc.vector.tensor_copy(CC[:], ps[:])
                    for t in range(8):
                        qe = nc.gpsimd if t % 2 == 0 else nc.sync
                        qe.dma_start(
                            T[0:16, 16 * t:16 * t + 16],
                            CC[t:128:8, t:128:8])
                    nc.vector.tensor_reduce(
                        M[:], T[0:16, :].rearrange("p (t n) -> p n t", t=8),
                        mybir.AxisListType.X, AL.add)
                    # G[0, 256*ai + 128*m + 16*h + l] = M[l, 8*m + h]
                    for m in range(2):
                        for h in range(8):
                            s = 256 * ai + 128 * m + 16 * h
                            qe = nc.gpsimd if h % 2 == 0 else nc.sync
                            qe.dma_start(
                                G[0:1, s:s + 16],
                                M[0:16, 8 * m + h:8 * m + h + 1])
                    nc.gpsimd.dma_start(
                        cc_h_in[ai][:], G[0:1, 256 * ai:256 * ai + 256])
                    nc.gpsimd.collective_compute(
                        "AllReduce", AL.add, replica_groups=[core_ids],
                        ins=[cc_h_in[ai].opt()], outs=[cc_h_out[ai].opt()],
                    )
                    nc.gpsimd.dma_start(
                        GH[0:1, 256 * ai:256 * ai + 256], cc_h_out[ai][:])

                # interleave sim/exp chunks; exp gets a 3-chunk head start
                # so its extraction + allreduce hide under sim's tail
                HS = 3
                seq = [(1, ci) for ci in range(HS)]
                si, ei = 0, HS
                while si < NCH or ei < NCH:
                    if si < NCH:
                        seq.append((0, si)); si += 1
                    if ei < NCH:
                        seq.append((1, ei)); ei += 1
                done = [0, 0]
                for ai, ci in seq:
                    arr = sim_ext if ai == 0 else exp_ext
                    weighted = ai == 0
                    ps = ps_t[ai]
                    if True:
                        c0 = ci * FC
                        nc.sync.dma_start(x[:], arr[:, c0:c0 + FC])
                        if weighted:
                            nc.sync.dma_start(wt[:], w_ext[:, c0:c0 + FC])
                            nc.scalar.activation(
                                wb[:], wt[:], ACT.Identity, bias=0.0,
                                scale=1.0)
                        # u = x*inv + b_u   (Scalar engine)
                        nc.scalar.activation(
                            u[:], x[:], ACT.Identity,
                            bias=bc[:, 1:2], scale=bc[:, 0:1])
                        # kc = clip(round(u-0.5), 0, 126)  [fused magic]
                        nc.vector.tensor_scalar(
                            kcm[:], u[:], MHALF, MAGIC + 126.0,
                            AL.add, AL.min)
                        nc.vector.tensor_scalar(
                            kcc[:], kcm[:], -MAGIC, 0.0, AL.add, AL.max)
                        # frac = u - kc  (bf16)
                        nc.vector.tensor_tensor(
                            frac[:], u[:], kcc[:], AL.subtract)
                        # kh = round(kc/16 - 0.499) ; kl = kc - 16*kh
                        nc.vector.tensor_scalar(
                            kcm[:], kcc[:], 0.0625, -0.499, AL.mult, AL.add)
                        nc.vector.tensor_scalar(
                            khb[:], kcm[:], 12582912.0, -12582912.0,
                            AL.add, AL.add)
                        nc.vector.scalar_tensor_tensor(
                            klb[:], khb[:], -16.0, kcc[:], AL.mult, AL.add)
                        # planes + matmuls in two half-chunks so the PE
                        # consumes half A while DVE builds half B
                        nal = 7
                        for hf in range(2):
                            ALg = ALgh[hf]
                            RHg = RHgh[hf]
                            sl = slice(hf * FH, (hf + 1) * FH)
                            for l in range(nal):
                                nc.scalar.activation(
                                    ab8[:, l * FC + hf * FH:
                                        l * FC + hf * FH + FH], klb[:, sl],
                                    ACT.Abs, bias=lbias[:, l:l + 1],
                                    scale=1.0)
                            for l in range(nal):
                                nc.scalar.activation(
                                    ALg[:, l],
                                    ab8[:, l * FC + hf * FH:
                                        l * FC + hf * FH + FH],
                                    ACT.Relu, bias=hbias[:, 8:9], scale=-1.0)
                            for l in range(nal, 16):
                                nc.vector.tensor_scalar(
                                    ALg[:, l], klb[:, sl],
                                    float(l), 1.0, AL.is_equal, AL.mult)
                            for h in range(8):
                                nc.vector.tensor_scalar(
                                    RHg[:, h], khb[:, sl],
                                    float(h), 1.0, AL.is_equal, AL.mult)
                            if weighted:
                                for h in range(8):
                                    nc.vector.tensor_tensor(
                                        RHg[:, h], RHg[:, h], wb[:, sl],
                                        AL.mult)
                            for h in range(8):
                                nc.vector.tensor_tensor(
                                    RHg[:, 8 + h], RHg[:, h], frac[:, sl],
                                    AL.mult)
                            for g in range(0, FH, 8):
                                first = ci == 0 and hf == 0 and g == 0
                                last = (ci == NCH - 1 and hf == 1
                                        and g == FH - 8)
                                nc.tensor.matmul(
                                    ps[:], AL16h[hf][:, g * 16:g * 16 + 128],
                                    RHh[hf][:, g * 16:g * 16 + 128],
                                    start=first, stop=last,
                                    skip_group_check=True)
                    done[ai] += 1
                    if done[ai] == NCH:
                        extract(ai)

            # ---------------- Phase C: all-reduce + chi2 ----------------
            with tc.tile_pool(name="pc", bufs=1) as pc:
                R = pc.tile([1, 256], F32, name="R")
                nc.vector.memset(R[:], 0.0)
                for ai in range(2):
                    M0 = GH[0:1, 256 * ai:256 * ai + 128]
                    M1 = GH[0:1, 256 * ai + 128:256 * ai + 256]
                    Ra = R[0:1, 128 * ai:128 * ai + 128]
                    # raw[b] = M1[b-1] + M0[b] - M1[b],  b in 1..126
                    nc.vector.tensor_tensor(
                        Ra[0:1, 1:127], M0[0:1, 1:127], M1[0:1, 1:127],
                        AL.subtract)
                    nc.vector.tensor_tensor(
                        Ra[0:1, 1:127], Ra[0:1, 1:127], M1[0:1, 0:126],
                        AL.add)
                    ssum = pc.tile([1, 1], F32, name=f"ssum{ai}")
                    nc.vector.tensor_reduce(
                        ssum[:], Ra, mybir.AxisListType.X, AL.add)
                    nc.vector.reciprocal(ssum[:], ssum[:])
                    nc.vector.tensor_scalar(
                        Ra, Ra, ssum[0:1, 0:1], None, AL.mult)
                dif = pc.tile([1, BINS], F32, name="dif")
                nc.vector.tensor_tensor(
                    dif[:], R[0:1, 0:128], R[0:1, 128:256], AL.subtract)
                nc.vector.tensor_tensor(dif[:], dif[:], dif[:], AL.mult)
                chi = pc.tile([1, 1], F32, name="chi")
                nc.vector.tensor_reduce(
                    chi[:], dif[:], mybir.AxisListType.X, AL.add)
                # * (1/delta)^2
                nc.vector.tensor_scalar(
                    chi[:], chi[:], sc[0:1, 3:4], None, AL.mult)
                nc.gpsimd.dma_start(out_ext[:], chi[:])

    _split_sync_waits(nc, __import__("concourse.mybir", fromlist=["x"]),
                      strip_same_engine=True)
    return nc


_CACHE = {}


def _get_nc(repeat=1):
    key = (repeat,)
    if key not in _CACHE:
        _CACHE[key] = build()
    return _CACHE[key]


def make_in_maps(sim, exp, w):
    sim_s = sim.reshape(NCORES, P, FTOT)
    exp_s = exp.reshape(NCORES, P, FTOT)
    w_s = w.reshape(NCORES, P, FTOT)
    return [
        {"sim": sim_s[c], "exp": exp_s[c], "w": w_s[c]} for c in range(NCORES)
    ]


def finish(res):
    val = res.results[0]["out"][0, 0]
    return np.asarray(val, dtype=np.float32).reshape(())


def kernel(**inputs):
    sim = np.ascontiguousarray(inputs["sim_observable"], dtype=np.float32)
    exp = np.ascontiguousarray(inputs["exp_observable"], dtype=np.float32)
    w = np.ascontiguousarray(inputs["weights"], dtype=np.float32)
    assert sim.shape == (N,) and exp.shape == (N,) and w.shape == (N,)

    from concourse.bass_utils import run_bass_kernel_spmd

    repeat = int(os.environ.get("BASS_HIST_REPEAT", "1"))
    nc = _get_nc(repeat)
    res = run_bass_kernel_spmd(nc, make_in_maps(sim, exp, w),
                               list(range(NCORES)))
    return finish(res)


# revision 39
# speedup vs baseline: 1.2632x; 1.0234x over previous
"""Trainium2 Bass kernel for nn_BinnedLoss (tent-weighted 128-bin chi2 loss).

Self-contained 8-core SPMD program, data-parallel over the N=16.7M sample
axis. Per core each array is [128, 16384] f32. The tent-weighted histogram
is fully determined by per-interval moments M0[k]=sum(w), M1[k]=sum(w*frac)
(k = clipped floor((x-mn)/step), frac = u-k), so instead of a per-bin
mask-reduce loop the kernel:
  * computes u on the Scalar engine, kc/kh/kl/frac via fused 2x/4x-mode
    tensor_scalar ops on DVE,
  * builds planar one-hot factors (kc = 16*kh + kl) in bf16 with 4x-mode
    is_equal tensor_scalar ops,
  * accumulates per-cell moments with block-diagonal [128,128] bf16 matmuls
    on the Tensor engine into PSUM (8 sample-columns per matmul; both
    operands contiguous via a grouped-interleaved plane layout, since the
    stationary operand must be a 1-free-dim AP and a multi-dim moving AP
    streams ~6x slower; plane tiles are split into two half-chunk pairs so
    the PE consumes half A while DVE builds half B),
  * offloads part of the one-hot builds to the Scalar engine as
    Relu(1 - |k - c|) activation pairs,
  * extracts the 8 diagonal [16,16] blocks, allreduces one [1,256] vector
    per array (the sim one hides under exp compute), and assembles
    raw[b] = M1[b-1] + M0[b] - M1[b] (b in 1..126) before the
    normalization and chi2 reduction.

kernel(**inputs) -> np.float32 scalar (shape ()).
"""
import os
import sys

sys.path.insert(0, "/opt/trn_rl_repo")
import numpy as np

N = 16777216
NCORES = 8
BINS = 128
P = 128
NSH = N // NCORES            # samples per core
FTOT = NSH // P              # 16384 free columns per core per array
FC = 1024                    # columns per phase-B chunk
NCH = FTOT // FC
MAGIC = 8388608.0            # 2^23
MHALF = 8388607.5            # 2^23 - 0.5 (round(u-0.5) == floor for our grid)


def _patches(mybir, tile):
    from concourse.vector_clock import ScopedClock

    def _patched(self, tick_clock, wait_clock):
        drain_inst = self.nc.sync.drain()
        wait_clock.add_sem_waits(
            drain_inst.ins, ScopedClock({None: tick_clock.global_clock})
        )
        si = drain_inst.ins.sync_info
        if si is not None and si.on_wait and len(si.on_wait) > 1:
            waits = list(si.on_wait)
            drain_inst.ins.sync_info = mybir.SyncInfo(
                on_wait=[waits[0]], on_update=list(si.on_update)
            )
            for w in waits[1:]:
                nop = self.nc.sync.nop()
                nop.ins.sync_info = mybir.SyncInfo(on_wait=[w], on_update=[])
        self.nc.all_engine_barrier()
        assert self.sems is not None
        popped = self.nc._tile_sem_poison_stack.pop()
        assert popped is self._sem_poison
        self.nc.clear_and_free_semaphores(list(self.sems.allocated().values()))
        self.nc.all_engine_barrier()

    tile.TileContext._drain_and_barrier = _patched


def _split_sync_waits(nc, mybir, strip_same_engine=True):
    """Two fixups for this walrus/runtime:
    1. Drop same-engine waits (engines are in-order; engine-vs-own-sem waits
       are redundant) -- wait-carrying instructions are ~10x slower here.
    2. The walrus build allows <=1 sem-wait per instruction; hoist extras
       onto same-engine NOPs inserted just before the instruction."""
    eng_sem = {}
    counter = [0]
    for f in nc.m.functions:
        for bb in f.blocks:
            out = []
            dirty = False
            for inst in bb.instructions:
                si = inst.sync_info
                pref = eng_sem.get(inst.engine) if strip_same_engine else None
                if si is not None and si.on_wait and pref is not None:
                    kept = [
                        w for w in si.on_wait
                        if not (w.ant_name or "").startswith(pref + "_")
                    ]
                    if len(kept) != len(si.on_wait):
                        inst.sync_info = mybir.SyncInfo(
                            on_wait=kept, on_update=list(si.on_update))
                        si = inst.sync_info
                        dirty = True
                if si is not None and si.on_wait and len(si.on_wait) > 1:
                    waits = list(si.on_wait)
                    for w in waits[:-1]:
                        counter[0] += 1
                        nop = mybir.InstNoOp(
                            name=f"WSPLIT-{counter[0]}", ins=[], outs=[]
                        )
                        nop.engine = inst.engine
                        nop.sync_info = mybir.SyncInfo(on_wait=[w], on_update=[])
                        nc.register_instruction(nop, overwrite=True)
                        out.append(nop)
                    inst.sync_info = mybir.SyncInfo(
                        on_wait=[waits[-1]], on_update=list(si.on_update)
                    )
                    dirty = True
                out.append(inst)
            if dirty:
                bb.instructions = out


def build(ncores=NCORES):
    import concourse.bass as bass
    import concourse.mybir as mybir
    from concourse import tile

    _patches(mybir, tile)
    DT = mybir.dt
    AL = mybir.AluOpType
    ACT = mybir.ActivationFunctionType
    F32 = DT.float32
    BF16 = DT.bfloat16
    core_ids = list(range(ncores))

    nc = bass.Bass()
    sim_ext = nc.declare_dram_parameter("sim", [P, FTOT], F32, isOutput=False)
    exp_ext = nc.declare_dram_parameter("exp", [P, FTOT], F32, isOutput=False)
    w_ext = nc.declare_dram_parameter("w", [P, FTOT], F32, isOutput=False)
    out_ext = nc.declare_dram_parameter("out", [1, 1], F32, isOutput=True)

    with tile.TileContext(nc) as tc:
        with (
            tc.tile_pool(name="const", bufs=1) as cpool,
            tc.tile_pool(name="dram", bufs=1, space="DRAM") as dram,
            tc.tile_pool(name="psum", bufs=1, space="PSUM") as psum,
        ):
            cc_a_in = dram.tile([1, 2], F32, name="cc_a_in")
            cc_a_out = dram.tile([1, 2], F32, name="cc_a_out")
            cc_h_in = [dram.tile([1, 256], F32, name=f"cc_h_in{a}")
                       for a in range(2)]
            cc_h_out = [dram.tile([1, 256], F32, name=f"cc_h_out{a}")
                        for a in range(2)]

            ones1 = cpool.tile([1, P], F32, name="ones1")
            nc.vector.memset(ones1[:], 1.0)

            # scalars: sc = [mn, inv, b_u, idel2] (+ scratch)
            sc = cpool.tile([1, 8], F32, name="sc")
            bc = cpool.tile([P, 2], F32, name="bc")
            bcps = psum.tile([P, 2], F32, name="bcps", tag="bcps")

            # per-array moment rows [M0 | M1] c-ordered, both arrays
            G = cpool.tile([1, 512], F32, name="G")
            GH = cpool.tile([1, 512], F32, name="GH")

            # ---------------- Phase A: global min/max ----------------
            with tc.tile_pool(name="pa", bufs=1) as pa:
                CW = 8192
                rmin = pa.tile([P, 1], F32, name="rmin")
                rmax = pa.tile([P, 1], F32, name="rmax")
                nc.vector.memset(rmin[:], 1.0e30)
                nc.vector.memset(rmax[:], -1.0e30)
                ch = pa.tile([P, CW], F32, name="ch", bufs=4)
                tmin = pa.tile([P, 1], F32, name="tmin")
                tmax = pa.tile([P, 1], F32, name="tmax")
                H = CW // 2
                for arr in (sim_ext, exp_ext):
                    for cv in range(0, FTOT, CW):
                        nc.sync.dma_start(ch[:, 0:H], arr[:, cv:cv + H])
                        nc.scalar.dma_start(
                            ch[:, H:CW], arr[:, cv + H:cv + CW])
                        nc.vector.tensor_reduce(
                            tmin[:], ch[:], mybir.AxisListType.X, AL.min)
                        nc.vector.tensor_reduce(
                            tmax[:], ch[:], mybir.AxisListType.X, AL.max)
                        nc.vector.tensor_tensor(
                            rmin[:], rmin[:], tmin[:], AL.min)
                        nc.vector.tensor_tensor(
                            rmax[:], rmax[:], tmax[:], AL.max)
                pm = pa.tile([1, 2 * P], F32, name="pm")
                nc.gpsimd.dma_start(pm[0:1, 0:P], rmax[:, 0:1])
                nc.gpsimd.dma_start(pm[0:1, P:2 * P], rmin[:, 0:1])
                pk = pa.tile([1, 2], F32, name="pk")
                nc.vector.tensor_reduce(
                    pk[0:1, 0:1], pm[0:1, 0:P], mybir.AxisListType.X, AL.max)
                nc.vector.tensor_reduce(
                    pk[0:1, 1:2], pm[0:1, P:2 * P], mybir.AxisListType.X,
                    AL.min)
                nc.vector.tensor_scalar_mul(pk[0:1, 1:2], pk[0:1, 1:2], -1.0)
                nc.gpsimd.dma_start(cc_a_in[:], pk[:])
                nc.gpsimd.collective_compute(
                    "AllReduce", AL.max, replica_groups=[core_ids],
                    ins=[cc_a_in.opt()], outs=[cc_a_out.opt()],
                )
                ga = pa.tile([1, 2], F32, name="ga")
                nc.gpsimd.dma_start(ga[:], cc_a_out[:])
                # ga = [mx, -mn]
                nc.vector.tensor_scalar_mul(sc[0:1, 0:1], ga[0:1, 1:2], -1.0)
                d_t = pa.tile([1, 1], F32, name="d_t")
                nc.vector.tensor_tensor(
                    d_t[:], ga[0:1, 0:1], sc[0:1, 0:1], AL.subtract)
                # step = d/127 ; inv = 1/step ; b_u = -mn*inv
                stp = pa.tile([1, 1], F32, name="stp")
                nc.vector.tensor_scalar_mul(
                    stp[:], d_t[:], float(np.float32(1.0) / np.float32(127.0)))
                nc.vector.reciprocal(sc[0:1, 1:2], stp[:])
                nc.vector.scalar_tensor_tensor(
                    sc[0:1, 2:3], sc[0:1, 0:1], -1.0, sc[0:1, 1:2],
                    AL.mult, AL.mult)
                # idel2 = (1/delta)^2, delta = d/128
                dl = pa.tile([1, 1], F32, name="dl")
                nc.vector.tensor_scalar_mul(dl[:], d_t[:], 0.0078125)
                nc.vector.reciprocal(dl[:], dl[:])
                nc.vector.tensor_tensor(sc[0:1, 3:4], dl[:], dl[:], AL.mult)
                # broadcast [inv, b_u] to all partitions
                nc.tensor.matmul(bcps[:], ones1[:], sc[0:1, 1:3],
                                 start=True, stop=True)
                nc.vector.tensor_copy(bc[:], bcps[:])

            # ---------------- Phase B: moment accumulation ----------------
            with tc.tile_pool(name="pb", bufs=1) as pb:
                x = pb.tile([P, FC], F32, name="x", bufs=3)
                u = pb.tile([P, FC], F32, name="u", bufs=2)
                kcm = pb.tile([P, FC], F32, name="kcm")
                kcc = pb.tile([P, FC], F32, name="kcc")
                frac = pb.tile([P, FC], BF16, name="frac", bufs=2)
                khb = pb.tile([P, FC], BF16, name="khb", bufs=2)
                klb = pb.tile([P, FC], BF16, name="klb", bufs=2)
                FH = FC // 2
                AL16h = [pb.tile([P, 16 * FH], BF16, name=f"AL16{i}",
                                 bufs=2) for i in range(2)]
                RHh = [pb.tile([P, 16 * FH], BF16, name=f"RH{i}", bufs=2)
                       for i in range(2)]
                wt = pb.tile([P, FC], F32, name="wt", bufs=3)
                wb = pb.tile([P, FC], BF16, name="wb", bufs=2)
                ab8 = pb.tile([P, 8 * FC], BF16, name="ab8")
                hbias = pb.tile([P, 9], F32, name="hbias")
                lbias = pb.tile([P, 7], F32, name="lbias")
                for l in range(7):
                    nc.vector.memset(lbias[:, l:l + 1], -float(l))
                for h in range(8):
                    nc.vector.memset(hbias[:, h:h + 1], -float(h))
                nc.vector.memset(hbias[:, 8:9], 1.0)
                CC = pb.tile([P, 128], F32, name="CC", bufs=2)
                T = pb.tile([16, 128], F32, name="T", bufs=2)
                M = pb.tile([16, 16], F32, name="M", bufs=2)
                # grouped-interleave: mem idx = g*128 + l*8 + t (per half)
                ALgh = [t_[:].rearrange("p (g l t) -> p l g t", l=16, t=8)
                        for t_ in AL16h]
                RHgh = [t_[:].rearrange("p (g n t) -> p n g t", n=16, t=8)
                        for t_ in RHh]
                ps_t = [psum.tile([P, 128], F32, name=f"ps{a}",
                                  tag=f"ps{a}") for a in range(2)]

                def extract(ai):
                    ps = ps_t[ai]
                    # extract: psum cell ps[8l+t, 8*(m*8+h)+t]
                    nc.vector.tensor_copy(CC[:], ps[:])
                    for t in range(8):
                        qe = nc.gpsimd if t % 2 == 0 else nc.sync
                        qe.dma_start(
                            T[0:16, 16 * t:16 * t + 16],
                            CC[t:128:8, t:128:8])
                    nc.vector.tensor_reduce(
                        M[:], T[0:16, :].rearrange("p (t n) -> p n t", t=8),
                        mybir.AxisListType.X, AL.add)
                    # G[0, 256*ai + 128*m + 16*h + l] = M[l, 8*m + h]
                    for m in range(2):
                        for h in range(8):
                            s = 256 * ai + 128 * m + 16 * h
                            qe = nc.gpsimd if h % 2 == 0 else nc.sync
                            qe.dma_start(
                                G[0:1, s:s + 16],
                                M[0:16, 8 * m + h:8 * m + h + 1])
                    nc.gpsimd.dma_start(
                        cc_h_in[ai][:], G[0:1, 256 * ai:256 * ai + 256])
                    nc.gpsimd.collective_compute(
                        "AllReduce", AL.add, replica_groups=[core_ids],
                        ins=[cc_h_in[ai].opt()], outs=[cc_h_out[ai].opt()],
                    )
                    nc.gpsimd.dma_start(
                        GH[0:1, 256 * ai:256 * ai + 256], cc_h_out[ai][:])

                # interleave sim/exp chunks; exp gets a 3-chunk head start
                # so its extraction + allreduce hide under sim's tail
                HS = 3
                seq = [(1, ci) for ci in range(HS)]
                si, ei = 0, HS
                while si < NCH or ei < NCH:
                    if si < NCH:
                        seq.append((0, si)); si += 1
                    if ei < NCH:
                        seq.append((1, ei)); ei += 1
                done = [0, 0]
                for ai, ci in seq:
                    arr = sim_ext if ai == 0 else exp_ext
                    weighted = ai == 0
                    ps = ps_t[ai]
                    if True:
                        c0 = ci * FC
                        nc.sync.dma_start(x[:], arr[:, c0:c0 + FC])
                        if weighted:
                            nc.scalar.dma_start(wt[:], w_ext[:, c0:c0 + FC])
                            nc.scalar.activation(
                                wb[:], wt[:], ACT.Identity, bias=0.0,
                                scale=1.0)
                        # u = x*inv + b_u   (Scalar engine)
                        nc.scalar.activation(
                            u[:], x[:], ACT.Identity,
                            bias=bc[:, 1:2], scale=bc[:, 0:1])
                        # kc = clip(round(u-0.5), 0, 126)  [fused magic]
                        nc.vector.tensor_scalar(
                            kcm[:], u[:], MHALF, MAGIC + 126.0,
                            AL.add, AL.min)
                        nc.vector.tensor_scalar(
                            kcc[:], kcm[:], -MAGIC, 0.0, AL.add, AL.max)
                        # frac = u - kc  (bf16)
                        nc.vector.tensor_tensor(
                            frac[:], u[:], kcc[:], AL.subtract)
                        # kh = round(kc/16 - 0.499) ; kl = kc - 16*kh
                        nc.vector.tensor_scalar(
                            kcm[:], kcc[:], 0.0625, -0.499, AL.mult, AL.add)
                        nc.vector.tensor_scalar(
                            khb[:], kcm[:], 12582912.0, -12582912.0,
                            AL.add, AL.add)
                        nc.vector.scalar_tensor_tensor(
                            klb[:], khb[:], -16.0, kcc[:], AL.mult, AL.add)
                        # planes + matmuls in two half-chunks so the PE
                        # consumes half A while DVE builds half B
                        nal = 6
                        for hf in range(2):
                            ALg = ALgh[hf]
                            RHg = RHgh[hf]
                            sl = slice(hf * FH, (hf + 1) * FH)
                            for l in range(nal):
                                nc.scalar.activation(
                                    ab8[:, l * FC + hf * FH:
                                        l * FC + hf * FH + FH], klb[:, sl],
                                    ACT.Abs, bias=lbias[:, l:l + 1],
                                    scale=1.0)
                            for l in range(nal):
                                nc.scalar.activation(
                                    ALg[:, l],
                                    ab8[:, l * FC + hf * FH:
                                        l * FC + hf * FH + FH],
                                    ACT.Relu, bias=hbias[:, 8:9], scale=-1.0)
                            for l in range(nal, 16):
                                nc.vector.tensor_scalar(
                                    ALg[:, l], klb[:, sl],
                                    float(l), 1.0, AL.is_equal, AL.mult)
                            for h in range(8):
                                nc.vector.tensor_scalar(
                                    RHg[:, h], khb[:, sl],
                                    float(h), 1.0, AL.is_equal, AL.mult)
                            if weighted:
                                for h in range(8):
                                    nc.vector.tensor_tensor(
                                        RHg[:, h], RHg[:, h], wb[:, sl],
                                        AL.mult)
                            for h in range(8):
                                nc.vector.tensor_tensor(
                                    RHg[:, 8 + h], RHg[:, h], frac[:, sl],
                                    AL.mult)
                            for g in range(0, FH, 8):
                                first = ci == 0 and hf == 0 and g == 0
                                last = (ci == NCH - 1 and hf == 1
                                        and g == FH - 8)
                                nc.tensor.matmul(
                                    ps[:], AL16h[hf][:, g * 16:g * 16 + 128],
                                    RHh[hf][:, g * 16:g * 16 + 128],
                                    start=first, stop=last,
                                    skip_group_check=True)
                    done[ai] += 1
                    if done[ai] == NCH:
                        extract(ai)

            # ---------------- Phase C: all-reduce + chi2 ----------------
            with tc.tile_pool(name="pc", bufs=1) as pc:
                R = pc.tile([1, 256], F32, name="R")
                nc.vector.memset(R[:], 0.0)
                for ai in range(2):
                    M0 = GH[0:1, 256 * ai:256 * ai + 128]
                    M1 = GH[0:1, 256 * ai + 128:256 * ai + 256]
                    Ra = R[0:1, 128 * ai:128 * ai + 128]
                    # raw[b] = M1[b-1] + M0[b] - M1[b],  b in 1..126
                    nc.vector.tensor_tensor(
                        Ra[0:1, 1:127], M0[0:1, 1:127], M1[0:1, 1:127],
                        AL.subtract)
                    nc.vector.tensor_tensor(
                        Ra[0:1, 1:127], Ra[0:1, 1:127], M1[0:1, 0:126],
                        AL.add)
                    ssum = pc.tile([1, 1], F32, name=f"ssum{ai}")
                    nc.vector.tensor_reduce(
                        ssum[:], Ra, mybir.AxisListType.X, AL.add)
                    nc.vector.reciprocal(ssum[:], ssum[:])
                    nc.vector.tensor_scalar(
                        Ra, Ra, ssum[0:1, 0:1], None, AL.mult)
                dif = pc.tile([1, BINS], F32, name="dif")
                nc.vector.tensor_tensor(
                    dif[:], R[0:1, 0:128], R[0:1, 128:256], AL.subtract)
                nc.vector.tensor_tensor(dif[:], dif[:], dif[:], AL.mult)
                chi = pc.tile([1, 1], F32, name="chi")
                nc.vector.tensor_reduce(
                    chi[:], dif[:], mybir.AxisListType.X, AL.add)
                # * (1/delta)^2
                nc.vector.tensor_scalar(
                    chi[:], chi[:], sc[0:1, 3:4], None, AL.mult)
                nc.gpsimd.dma_start(out_ext[:], chi[:])

    _split_sync_waits(nc, __import__("concourse.mybir", fromlist=["x"]),
                      strip_same_engine=True)
    return nc


_CACHE = {}


def _get_nc(repeat=1):
    key = (repeat,)
    if key not in _CACHE:
        _CACHE[key] = build()
    return _CACHE[key]


def make_in_maps(sim, exp, w):
    sim_s = sim.reshape(NCORES, P, FTOT)
    exp_s = exp.reshape(NCORES, P, FTOT)
    w_s = w.reshape(NCORES, P, FTOT)
    return [
        {"sim": sim_s[c], "exp": exp_s[c], "w": w_s[c]} for c in range(NCORES)
    ]


def finish(res):
    val = res.results[0]["out"][0, 0]
    return np.asarray(val, dtype=np.float32).reshape(())


def kernel(**inputs):
    sim = np.ascontiguousarray(inputs["sim_observable"], dtype=np.float32)
    exp = np.ascontiguousarray(inputs["exp_observable"], dtype=np.float32)
    w = np.ascontiguousarray(inputs["weights"], dtype=np.float32)
    assert sim.shape == (N,) and exp.shape == (N,) and w.shape == (N,)

    from concourse.bass_utils import run_bass_kernel_spmd

    repeat = int(os.environ.get("BASS_HIST_REPEAT", "1"))
    nc = _get_nc(repeat)
    res = run_bass_kernel_spmd(nc, make_in_maps(sim, exp, w),
                               list(range(NCORES)))
    return finish(res)
